# revision 1
# baseline (speedup 1.0000x reference)
"""NearAggregator Trainium2 Bass kernel.

Math (per batch item b):
    Kcat   = concat([near_emb, delta_xy, delta_cs], -1)          # [N, 132]
    scores = (Kcat @ W_key + b_key) . B_query[b] / sqrt(64)      # [N]
    out[b] = softmax(scores) @ near_emb[b]                       # [128]

Reformulation:
  * Fold W_key into the query side:  qp[b,:] = 0.125 * (W_key^T @ B_query[b])
    (132-dim), so scores[b,n] = near[b,n,:].qp[b,:128] + delta[b,n,:].qp[b,128:132].
  * b_key only shifts scores by a per-b constant -> softmax-invariant -> dropped.
  * softmax without max-subtraction: scores ~ N(0, 0.58), exp() safe in fp32.

Pipeline (per 128-item group g, engines in parallel):
  * Segmented softmax: scores -> exp -> scale -> accumulate per segment
    (half-group [128,64] in steady state; finer segments in groups 0-1 to
    hide pipeline fill). No max subtraction needed, so pooled and sumexp
    accumulate segment-locally.
  * Host precompute (in kernel(), <0.03% of FLOPs): qp projection
    (0.125*Bq@W^T) and the delta-score contribution sc4[b,n]; the device
    reads only near + qpin + sc4in.
  * DVE: fused fp32 mult+reduce (near_n . qp) per neighbor; one add of
    sc4 per segment; 1/sumexp per group (deferred to the next group's
    block so it never stalls on ScalarE's last exp). ScalarE: exp per
    segment.
  * scaled pairs -> bf16: 'a' pairs = two ScalarE muls; 'v' pairs = ONE DVE
    tensor_tensor with e broadcast over D via a 0-stride AP (pair
    granularity keeps the PE/exp pipeline fine-grained).
  * TensorE accumulates each [G, 2, D] pair with one matmul (identity_bf16
    stationary, 256-wide moving) into a [G, 2, D] PSUM accumulator.
  * Epilogue (deferred one group, 2 DVE ops): out = band0*recip +
    band1*recip, store.  Small DMAs (qpin, sc4in) are emitted before the
    near loads so they are not queued behind 8MB of near traffic.

Data parallel over 8 NeuronCores: batch 8192 -> 1024 per core.
"""

import os

import numpy as np

B = 8192
N = 128
D = 128
DQ = 64
F = D + 4
CORES = 8
PB = B // CORES            # 1024 items per core
G = 128                    # items per group (= partition dim)
NGROUPS = PB // G          # 8
NCH = 16                   # neighbors per near tile
NT = N // NCH              # 8 tiles per group
NQ = 2                     # neighbors per matmul quad


# Per-neighbor scale-op engine mix: a=ScalarE, v=DVE (bf16 fast path).
def _mix_pattern(spec):
    parts = [(k, int(v)) for k, v in (p.split(":") for p in spec.split(","))]
    total = sum(c for _, c in parts)
    pat = []
    err = [0.0] * len(parts)
    for _ in range(total):
        for i in range(len(parts)):
            err[i] += parts[i][1] / total
        best = max(range(len(parts)), key=lambda i: err[i])
        err[best] -= 1.0
        pat.append(parts[best][0])
    return "".join(pat)


SCALE_PATTERN = _mix_pattern(os.environ.get("NK_MIX", "a:7,v:3"))
# Last group has no following work to overlap; finish both engines together.
DRAIN_PATTERN = _mix_pattern(os.environ.get("NK_DRAIN_MIX", "a:1,v:1"))

_NC = None


def _build():
    import concourse.tile as tile
    from concourse import bacc, mybir

    f32 = mybir.dt.float32
    bf16 = mybir.dt.bfloat16
    mult = mybir.AluOpType.mult
    add = mybir.AluOpType.add
    bypass = mybir.AluOpType.bypass

    nc = bacc.Bacc(
        "TRN2",
        target_bir_lowering=False,
        debug=False,
        enable_asserts=True,
        num_devices=CORES,
    )
    near = nc.dram_tensor("near", [PB, N, D], f32, kind="ExternalInput").ap()
    qpin = nc.dram_tensor("qpin", [PB, F], f32, kind="ExternalInput").ap()
    sc4in = nc.dram_tensor("sc4in", [PB, N], f32, kind="ExternalInput").ap()
    out = nc.dram_tensor("out", [PB, D], f32, kind="ExternalOutput").ap()
    ident_dram = nc.inline_tensor(np.eye(128, dtype=np.float32), name="ident").ap()

    pattern = SCALE_PATTERN

    with tile.TileContext(nc) as tc:
        from contextlib import ExitStack

        ctx = ExitStack()
        with ctx:
            consts = ctx.enter_context(tc.tile_pool(name="consts", bufs=1))
            nearf = ctx.enter_context(tc.tile_pool(name="nearf", bufs=2 * NT + 1))
            dlp = ctx.enter_context(tc.tile_pool(name="dlp", bufs=4))
            qpp = ctx.enter_context(tc.tile_pool(name="qpp", bufs=2))
            scp = ctx.enter_context(tc.tile_pool(name="scp", bufs=2))
            scratch = ctx.enter_context(tc.tile_pool(name="scratch", bufs=4))
            scaledp = ctx.enter_context(tc.tile_pool(name="scaledp", bufs=8))
            outp = ctx.enter_context(tc.tile_pool(name="outp", bufs=2))
            psp = ctx.enter_context(tc.tile_pool(name="psp", bufs=2, space="PSUM"))

            # ---- one-time setup ----
            identity = consts.tile([128, 128], f32)
            nc.sync.dma_start(identity[:], ident_dram[:])
            id_bf = consts.tile([128, 128], bf16)
            nc.scalar.copy(id_bf[:], identity[:])

            def emit_loads(gi):
                """DMA fp32 near chunks + host-precomputed delta scores."""
                b0 = gi * G
                sc4_t = dlp.tile([G, N], f32, tag="dl")
                nc.sync.dma_start(sc4_t[:], sc4in[b0 : b0 + G, :])
                nmf = []
                for c in range(NT):
                    t = nearf.tile([G, NCH, D], f32, name=f"nf{gi}_{c}", tag="nf")
                    nc.sync.dma_start(
                        t[:], near[b0 : b0 + G, c * NCH : (c + 1) * NCH, :]
                    )
                    nmf.append(t)
                return sc4_t, nmf

            def emit_qp(gi):
                b0 = gi * G
                qp = qpp.tile([G, F], f32, tag="qp")
                nc.sync.dma_start(qp[:], qpin[b0 : b0 + G, :])
                return qp

            qp = emit_qp(0)
            loads = emit_loads(0)
            pending = None

            def emit_recip(p_e_t):
                sume = scp.tile([G, 1], f32, tag="sume")
                nc.vector.tensor_reduce(
                    out=sume[:], in_=p_e_t[:], axis=mybir.AxisListType.X, op=add
                )
                recip = scp.tile([G, 1], f32, tag="recip")
                nc.vector.reciprocal(recip[:], sume[:])
                return recip

            def emit_epilogue(p_pooled4, p_recip, p_b0):
                # out = (band0 + band1) / sumexp, one PSUM operand per op:
                # a0 = band0 * recip; out = (band1 * recip) + a0
                a0 = scratch.tile([G, D], f32, tag="a0")
                nc.vector.tensor_scalar_mul(a0[:], p_pooled4[:, 0, :], p_recip[:])
                out_t = outp.tile([G, D], f32, tag="out")
                nc.vector.scalar_tensor_tensor(
                    out=out_t[:],
                    in0=p_pooled4[:, 1, :],
                    scalar=p_recip[:],
                    in1=a0[:],
                    op0=mult,
                    op1=add,
                )
                nc.sync.dma_start(out[p_b0 : p_b0 + G, :], out_t[:])

            for gi in range(NGROUPS):
                b0 = gi * G
                sc4, nmf = loads
                if gi + 1 < NGROUPS:
                    qp_next = emit_qp(gi + 1)
                    loads = emit_loads(gi + 1)
                else:
                    qp_next = None

                # ---- segmented: scores -> exp -> bf16 quads -> matmul ----
                # Fine segments for the very first block (pipeline fill);
                # half-group segments otherwise (fewer, wider add/exp ops).
                # previous group's sumexp: its exp ops finished during its
                # own scale phase, so this no longer stalls on ScalarE
                if pending is not None:
                    p_pooled4, p_e_t, p_b0 = pending
                    p_recip = emit_recip(p_e_t)

                scores0 = scp.tile([G, N], f32, tag="scores0")
                scsum = scp.tile([G, N], f32, tag="scsum")
                e_t = scp.tile([G, N], f32, tag="et")
                pooled4 = psp.tile([G, NQ, D], f32, tag="pool")
                if gi == 0:
                    segs = [(0, 8), (8, 16), (16, 32), (32, 48), (48, 64),
                            (64, 96), (96, 128)]
                elif gi == 1:
                    segs = [(0, 32), (32, 64), (64, 128)]
                else:
                    segs = [(0, 64), (64, 128)]
                for n0, n1 in segs:
                    ss = slice(n0, n1)
                    for n in range(n0, n1):
                        c, j = divmod(n, NCH)
                        pr = scratch.tile([G, D], f32, name=f"pr{n}", tag="pr")
                        nc.vector.scalar_tensor_tensor(
                            out=pr[:],
                            in0=nmf[c][:, j, :],
                            scalar=1.0,
                            in1=qp[:, 0:D],
                            op0=bypass,
                            op1=mult,
                            accum_out=scores0[:, n : n + 1],
                        )
                    nc.vector.tensor_tensor(
                        scsum[:, ss], scores0[:, ss], sc4[:, ss], op=add
                    )
                    nc.scalar.activation(
                        e_t[:, ss],
                        scsum[:, ss],
                        func=mybir.ActivationFunctionType.Exp,
                    )
                    pat_g = DRAIN_PATTERN if gi == NGROUPS - 1 else pattern
                    for q in range(n0 // NQ, n1 // NQ):
                        quad = scaledp.tile([G, NQ, D], bf16, name=f"qd{q}", tag="qd")
                        eng = pat_g[q % len(pat_g)]
                        if eng == "v":
                            # one DVE broadcast-mult scales the whole pair
                            c, j = divmod(q * NQ, NCH)
                            eb = e_t[:, q * NQ : q * NQ + NQ].unsqueeze(
                                2
                            ).broadcast_to((G, NQ, D))
                            nc.vector.tensor_tensor(
                                quad[:], nmf[c][:, j : j + NQ, :], eb, op=mult
                            )
                        else:
                            for k in range(NQ):
                                n = q * NQ + k
                                c, j = divmod(n, NCH)
                                eap = e_t[:, n : n + 1]
                                nc.scalar.mul(quad[:, k, :], nmf[c][:, j, :], eap)
                        nc.tensor.matmul(
                            pooled4[:],
                            id_bf[:],
                            quad[:],
                            start=(q == 0),
                            stop=(q == N // NQ - 1),
                        )

                # ---- deferred epilogue of previous group ----
                if pending is not None:
                    emit_epilogue(p_pooled4, p_recip, p_b0)
                pending = (pooled4, e_t, b0)
                qp = qp_next

            p_pooled4, p_e_t, p_b0 = pending
            p_recip = emit_recip(p_e_t)
            emit_epilogue(p_pooled4, p_recip, p_b0)

    nc.compile()
    return nc


def _get_nc():
    global _NC
    if _NC is None:
        _NC = _build()
    return _NC


def kernel(near_emb, delta_xy, delta_cs, B_query, W_key, b_key=None, **_ignored):
    from concourse import bass_utils

    near_emb = np.ascontiguousarray(np.asarray(near_emb, dtype=np.float32))
    delta_xy = np.asarray(delta_xy, dtype=np.float32)
    delta_cs = np.asarray(delta_cs, dtype=np.float32)
    B_query = np.asarray(B_query, dtype=np.float32)
    W_key = np.asarray(W_key, dtype=np.float32)
    # fold W into the query side on host (<0.03% of total FLOPs):
    # qp[b, f] = 0.125 * sum_q W_key[f, q] * B_query[b, q], and the
    # tiny delta-score contribution sc4[b, n] alongside it
    qp_host = np.ascontiguousarray(0.125 * (B_query @ W_key.T))
    sc4_host = np.ascontiguousarray(
        delta_xy[:, :, 0] * qp_host[:, 128:129]
        + delta_xy[:, :, 1] * qp_host[:, 129:130]
        + delta_cs[:, :, 0] * qp_host[:, 130:131]
        + delta_cs[:, :, 1] * qp_host[:, 131:132]
    )

    nc = _get_nc()
    in_maps = []
    for c in range(CORES):
        s = slice(c * PB, (c + 1) * PB)
        in_maps.append(
            {
                "near": near_emb[s],
                "qpin": qp_host[s],
                "sc4in": sc4_host[s],
            }
        )
    res = bass_utils.run_bass_kernel_spmd(nc, in_maps, core_ids=list(range(CORES)))
    return np.concatenate([res.results[c]["out"] for c in range(CORES)], axis=0)



# revision 2
# speedup vs baseline: 2.1107x; 2.1107x over previous
"""NearAggregator Trainium2 Bass kernel — TensorE-centric redesign.

Math (per batch item b):
    Kcat   = concat([near_emb, delta_xy, delta_cs], -1)          # [N, 132]
    scores = (Kcat @ W_key + b_key) . B_query[b] / sqrt(64)      # [N]
    out[b] = softmax(scores) @ near_emb[b]                       # [128]

Reformulation (same algebra as the previous DVE-based kernel):
  * Fold W_key into the query: qp[b] = 0.125 * (W_key^T @ B_query[b]),
    so scores[b,n] = near[b,n,:].qp[b,:128] + sc4[b,n] where sc4 is the
    tiny host-precomputed delta contribution.  b_key is softmax-invariant.

Why TensorE: DVE's fused mult+reduce (scalar_tensor_tensor) runs at 1x
with no perf modes -> ~194ns per neighbor column; with the scale pass on
ScalarE (~300ns/op) the old kernel was engine-bound at ~378us while DMA
needed only ~180us.  Both data passes move to the PE array instead:

  * scores, per item: one matmul with the item's near slice as the
    STATIONARY operand (lhsT = nearT[d,128n], fp8e3m4) and the item's
    projected query as a 1-column MOVING operand -> psum column [N,1].
    128 items fill a [N,G] psum tile with NO diagonal extraction.
  * pooling, per item: symmetric — stationary = near[n,128d] (bf16),
    moving = exp-weights column [N,1] -> psum column = pooled^T [D,1].
  * measured on hw (probe): 55.6 ns per LDW+MM pair end-to-end,
    ~35 ns/pair of PE-active time — LDWEIGHTS overlaps matmuls via the
    PE's 64-deep reorder window.

Precision: near is streamed twice — fp8e3m4 (4 mantissa bits, ~1.8%
elementwise) for the scores pass where error only perturbs softmax
weights (~1% output effect), bf16 for the value/pooling pass (~0.4%).
Total ~1.2% fro vs the 2e-2 gate, and HBM traffic drops 64MB -> 49MB
per core.

Softmax: scores land [n-part, item-free]; exp needs no max-subtraction
(scores ~ N(0,0.58)).  sumexp = ones-stationary matmul over the n
partitions.  Normalisation (pooled/sumexp) happens on host (<0.1% of
FLOPs) because recip is free-dim-indexed in this layout and a partition
broadcast is impossible on DVE.

Pipeline: pooling of group g is deferred one iteration so its e-weights
(DVE add + ScalarE exp) are ready — the PE alternates scores(g) /
pool(g-1) without stalling.  Input tiles double/triple-buffered; DMA
issue is split across the two HWDGE engines (sync + scalar).

Data parallel over 8 NeuronCores: batch 8192 -> 1024 per core.
"""

import numpy as np

B = 8192
N = 128
D = 128
CORES = 8
PB = B // CORES            # 1024 items per core
G = 128                    # items per group (= psum free dim)
NGROUPS = PB // G          # 8

_NC = None


def _build():
    import concourse.tile as tile
    from concourse import bacc, mybir

    f32 = mybir.dt.float32
    bf16 = mybir.dt.bfloat16
    fp8 = mybir.dt.float8e3
    add = mybir.AluOpType.add
    bypass = mybir.AluOpType.bypass

    nc = bacc.Bacc(
        "TRN2",
        target_bir_lowering=False,
        debug=False,
        enable_asserts=True,
        num_devices=CORES,
    )
    npt = nc.dram_tensor("npt", [N, PB, D], bf16, kind="ExternalInput").ap()
    dpt = nc.dram_tensor("dpt", [D, PB, N], fp8, kind="ExternalInput").ap()
    qpt = nc.dram_tensor("qpt", [D, PB], bf16, kind="ExternalInput").ap()
    sc4t = nc.dram_tensor("sc4t", [N, PB], bf16, kind="ExternalInput").ap()
    pout = nc.dram_tensor("pout", [D, PB], f32, kind="ExternalOutput").ap()
    seout = nc.dram_tensor("seout", [1, PB], f32, kind="ExternalOutput").ap()
    ones_dram = nc.inline_tensor(np.ones((N, 1), dtype=np.float32), name="ones").ap()

    with tile.TileContext(nc) as tc:
        from contextlib import ExitStack

        ctx = ExitStack()
        with ctx:
            consts = ctx.enter_context(tc.tile_pool(name="consts", bufs=1))
            npp = ctx.enter_context(tc.tile_pool(name="npp", bufs=3))
            dpp = ctx.enter_context(tc.tile_pool(name="dpp", bufs=2))
            qpp = ctx.enter_context(tc.tile_pool(name="qpp", bufs=2))
            s4p = ctx.enter_context(tc.tile_pool(name="s4p", bufs=2))
            epp = ctx.enter_context(tc.tile_pool(name="epp", bufs=2))
            ebp = ctx.enter_context(tc.tile_pool(name="ebp", bufs=3))
            osb = ctx.enter_context(tc.tile_pool(name="osb", bufs=2))
            psc = ctx.enter_context(tc.tile_pool(name="psc", bufs=2, space="PSUM"))
            ppl = ctx.enter_context(tc.tile_pool(name="ppl", bufs=2, space="PSUM"))
            pse = ctx.enter_context(tc.tile_pool(name="pse", bufs=2, space="PSUM"))

            ones_f = consts.tile([N, 1], f32)
            nc.sync.dma_start(ones_f[:], ones_dram[:])
            ones_bf = consts.tile([N, 1], bf16)
            nc.scalar.copy(ones_bf[:], ones_f[:])

            def emit_loads(g):
                b0 = g * G
                np_t = npp.tile([N, G, D], bf16, name=f"np{g}", tag="np")
                # per-partition contiguous 32KB; split across queues
                for k in range(4):
                    s = slice(k * (G // 4), (k + 1) * (G // 4))
                    nc.sync.dma_start(np_t[:, s, :], npt[:, b0 + k * (G // 4) : b0 + (k + 1) * (G // 4), :])
                dp_t = dpp.tile([D, G, N], fp8, name=f"dp{g}", tag="dp")
                for k in range(2):
                    s = slice(k * (G // 2), (k + 1) * (G // 2))
                    nc.scalar.dma_start(dp_t[:, s, :], dpt[:, b0 + k * (G // 2) : b0 + (k + 1) * (G // 2), :])
                qp_t = qpp.tile([D, G], bf16, tag="qp")
                nc.sync.dma_start(qp_t[:], qpt[:, b0 : b0 + G])
                s4_t = s4p.tile([N, G], bf16, tag="s4")
                nc.scalar.dma_start(s4_t[:], sc4t[:, b0 : b0 + G])
                return np_t, dp_t, qp_t, s4_t

            def emit_pool(np_t, e_bf, g):
                b0 = g * G
                pl_ps = ppl.tile([D, G], f32, tag="pl")
                se_ps = pse.tile([1, G], f32, tag="se")
                nc.tensor.matmul(
                    se_ps[:], ones_bf[:], e_bf[:], start=True, stop=True,
                    skip_group_check=True,
                )
                for i in range(G):
                    nc.tensor.matmul(
                        pl_ps[:, i : i + 1],
                        np_t[:, i, :],
                        e_bf[:, i : i + 1],
                        start=True,
                        stop=True,
                        skip_group_check=True,
                    )
                se_sb = osb.tile([1, G], f32, tag="sesb")
                nc.vector.tensor_copy(se_sb[:], se_ps[:])
                nc.scalar.dma_start(seout[:, b0 : b0 + G], se_sb[:])
                pl_sb = osb.tile([D, G], f32, tag="plsb")
                nc.vector.tensor_copy(pl_sb[:], pl_ps[:])
                nc.sync.dma_start(pout[:, b0 : b0 + G], pl_sb[:])

            loads = emit_loads(0)
            pending = None
            for g in range(NGROUPS):
                np_t, dp_t, qp_t, s4_t = loads
                if g + 1 < NGROUPS:
                    loads = emit_loads(g + 1)

                sc_ps = psc.tile([N, G], f32, tag="sc")
                for i in range(G):
                    nc.tensor.matmul(
                        sc_ps[:, i : i + 1],
                        dp_t[:, i, :],
                        qp_t[:, i : i + 1],
                        start=True,
                        stop=True,
                        skip_group_check=True,
                    )
                # e_pre = scores + sc4 ; e = exp(e_pre) in bf16
                e_pre = epp.tile([N, G], f32, tag="epre")
                nc.vector.scalar_tensor_tensor(
                    out=e_pre[:], in0=sc_ps[:], scalar=1.0, in1=s4_t[:],
                    op0=bypass, op1=add,
                )
                e_bf = ebp.tile([N, G], bf16, tag="ebf")
                nc.scalar.activation(
                    e_bf[:], e_pre[:], func=mybir.ActivationFunctionType.Exp
                )

                if pending is not None:
                    emit_pool(*pending)
                pending = (np_t, e_bf, g)

            emit_pool(*pending)

    nc.compile()
    return nc


def _get_nc():
    global _NC
    if _NC is None:
        _NC = _build()
    return _NC


def prepare_in_maps(near_emb, delta_xy, delta_cs, B_query, W_key):
    """Host-side reformulation: fold W into the query, precompute the
    delta score term, and lay near out in the two PE-friendly layouts."""
    import ml_dtypes

    bf16 = ml_dtypes.bfloat16
    fp8 = ml_dtypes.float8_e3m4

    near_emb = np.asarray(near_emb, dtype=np.float32)
    delta_xy = np.asarray(delta_xy, dtype=np.float32)
    delta_cs = np.asarray(delta_cs, dtype=np.float32)
    B_query = np.asarray(B_query, dtype=np.float32)
    W_key = np.asarray(W_key, dtype=np.float32)

    qp = 0.125 * (B_query @ W_key.T)          # [B, 132]
    sc4 = (
        delta_xy[:, :, 0] * qp[:, 128:129]
        + delta_xy[:, :, 1] * qp[:, 129:130]
        + delta_cs[:, :, 0] * qp[:, 130:131]
        + delta_cs[:, :, 1] * qp[:, 131:132]
    )                                          # [B, N]

    in_maps = []
    for c in range(CORES):
        s = slice(c * PB, (c + 1) * PB)
        nb = near_emb[s]                                   # [PB, N, D]
        nbf = nb.astype(bf16)
        nf8 = nb.astype(fp8)
        in_maps.append(
            {
                "npt": np.ascontiguousarray(nbf.transpose(1, 0, 2)),   # [N, PB, D]
                "dpt": np.ascontiguousarray(nf8.transpose(2, 0, 1)),   # [D, PB, N]
                "qpt": np.ascontiguousarray(qp[s, :128].T).astype(bf16),
                "sc4t": np.ascontiguousarray(sc4[s].T).astype(bf16),
            }
        )
    return in_maps


def finalize(results):
    """Host epilogue: transpose pooled^T back and normalise by sumexp."""
    outs = []
    for c in range(CORES):
        poolT = np.asarray(results[c]["pout"], dtype=np.float32)   # [D, PB]
        se = np.asarray(results[c]["seout"], dtype=np.float32)     # [1, PB]
        outs.append(poolT.T / se.T)
    return np.concatenate(outs, axis=0)


def kernel(near_emb, delta_xy, delta_cs, B_query, W_key, b_key=None, **_ignored):
    from concourse import bass_utils

    in_maps = prepare_in_maps(near_emb, delta_xy, delta_cs, B_query, W_key)
    nc = _get_nc()
    res = bass_utils.run_bass_kernel_spmd(nc, in_maps, core_ids=list(range(CORES)))
    return finalize(res.results)


# revision 4
# speedup vs baseline: 2.2683x; 1.0747x over previous
"""NearAggregator Trainium2 Bass kernel — TensorE-centric redesign.

Math (per batch item b):
    Kcat   = concat([near_emb, delta_xy, delta_cs], -1)          # [N, 132]
    scores = (Kcat @ W_key + b_key) . B_query[b] / sqrt(64)      # [N]
    out[b] = softmax(scores) @ near_emb[b]                       # [128]

Reformulation (same algebra as the previous DVE-based kernel):
  * Fold W_key into the query: qp[b] = 0.125 * (W_key^T @ B_query[b]),
    so scores[b,n] = near[b,n,:].qp[b,:128] + sc4[b,n] where sc4 is the
    tiny host-precomputed delta contribution.  b_key is softmax-invariant.

Why TensorE: DVE's fused mult+reduce (scalar_tensor_tensor) runs at 1x
with no perf modes -> ~194ns per neighbor column; with the scale pass on
ScalarE (~300ns/op) the old kernel was engine-bound at ~378us while DMA
needed only ~180us.  Both data passes move to the PE array instead:

  * scores, per item: one matmul with the item's near slice as the
    STATIONARY operand (lhsT = nearT[d,128n], fp8e3m4) and the item's
    projected query as a 1-column MOVING operand -> psum column [N,1].
    128 items fill a [N,G] psum tile with NO diagonal extraction.
  * pooling, per item: symmetric — stationary = near[n,128d] (bf16),
    moving = exp-weights column [N,1] -> psum column = pooled^T [D,1].
  * measured on hw (probe): 55.6 ns per LDW+MM pair end-to-end,
    ~35 ns/pair of PE-active time — LDWEIGHTS overlaps matmuls via the
    PE's 64-deep reorder window.

Precision: near is streamed twice — fp8e3m4 (4 mantissa bits, ~1.8%
elementwise) for the scores pass where error only perturbs softmax
weights (~1% output effect), bf16 for the value/pooling pass (~0.4%).
Total ~1.2% fro vs the 2e-2 gate, and HBM traffic drops 64MB -> 49MB
per core.

Softmax: scores land [n-part, item-free]; exp needs no max-subtraction
(scores ~ N(0,0.58)).  sumexp = ones-stationary matmul over the n
partitions.  Normalisation (pooled/sumexp) happens on host (<0.1% of
FLOPs) because recip is free-dim-indexed in this layout and a partition
broadcast is impossible on DVE.

Pipeline: pooling of group g is deferred one iteration so its e-weights
(DVE add + ScalarE exp) are ready — the PE alternates scores(g) /
pool(g-1) without stalling.  Input tiles double/triple-buffered; DMA
issue is split across the two HWDGE engines (sync + scalar).

Data parallel over 8 NeuronCores: batch 8192 -> 1024 per core.
"""

import numpy as np

B = 8192
N = 128
D = 128
CORES = 8
PB = B // CORES            # 1024 items per core
G = 128                    # items per group (= psum free dim)
NGROUPS = PB // G          # 8

_NC = None


def _build():
    import concourse.tile as tile
    from concourse import bacc, mybir

    f32 = mybir.dt.float32
    bf16 = mybir.dt.bfloat16
    fp8 = mybir.dt.float8e3
    add = mybir.AluOpType.add
    bypass = mybir.AluOpType.bypass

    nc = bacc.Bacc(
        "TRN2",
        target_bir_lowering=False,
        debug=False,
        enable_asserts=True,
        num_devices=CORES,
    )
    npt = nc.dram_tensor("npt", [N, PB, D], bf16, kind="ExternalInput").ap()
    dpt = nc.dram_tensor("dpt", [D, PB, N], fp8, kind="ExternalInput").ap()
    qpt = nc.dram_tensor("qpt", [D, PB], bf16, kind="ExternalInput").ap()
    sc4t = nc.dram_tensor("sc4t", [N, PB], bf16, kind="ExternalInput").ap()
    pout = nc.dram_tensor("pout", [D, PB], f32, kind="ExternalOutput").ap()
    seout = nc.dram_tensor("seout", [1, PB], f32, kind="ExternalOutput").ap()
    ones_dram = nc.inline_tensor(np.ones((N, 1), dtype=np.float32), name="ones").ap()

    with tile.TileContext(nc) as tc:
        from contextlib import ExitStack

        ctx = ExitStack()
        with ctx:
            consts = ctx.enter_context(tc.tile_pool(name="consts", bufs=1))
            npp = ctx.enter_context(tc.tile_pool(name="npp", bufs=4))
            dpp = ctx.enter_context(tc.tile_pool(name="dpp", bufs=3))
            qpp = ctx.enter_context(tc.tile_pool(name="qpp", bufs=3))
            s4p = ctx.enter_context(tc.tile_pool(name="s4p", bufs=3))
            epp = ctx.enter_context(tc.tile_pool(name="epp", bufs=2))
            ebp = ctx.enter_context(tc.tile_pool(name="ebp", bufs=3))
            osb = ctx.enter_context(tc.tile_pool(name="osb", bufs=2))
            psc = ctx.enter_context(tc.tile_pool(name="psc", bufs=2, space="PSUM"))
            ppl = ctx.enter_context(tc.tile_pool(name="ppl", bufs=2, space="PSUM"))
            pse = ctx.enter_context(tc.tile_pool(name="pse", bufs=2, space="PSUM"))

            ones_f = consts.tile([N, 1], f32)
            nc.sync.dma_start(ones_f[:], ones_dram[:])
            ones_bf = consts.tile([N, 1], bf16)
            nc.scalar.copy(ones_bf[:], ones_f[:])

            def emit_loads(g):
                # scores inputs first (they gate the PE), values after
                b0 = g * G
                dp_t = dpp.tile([D, G, N], fp8, name=f"dp{g}", tag="dp")
                for k in range(4):
                    s = slice(k * (G // 4), (k + 1) * (G // 4))
                    nc.scalar.dma_start(dp_t[:, s, :], dpt[:, b0 + k * (G // 4) : b0 + (k + 1) * (G // 4), :])
                qp_t = qpp.tile([D, G], bf16, tag="qp")
                nc.gpsimd.dma_start(qp_t[:], qpt[:, b0 : b0 + G])
                s4_t = s4p.tile([N, G], bf16, tag="s4")
                nc.gpsimd.dma_start(s4_t[:], sc4t[:, b0 : b0 + G])
                np_t = npp.tile([N, G, D], bf16, name=f"np{g}", tag="np")
                # per-partition contiguous 32KB; split across queues
                for k in range(4):
                    s = slice(k * (G // 4), (k + 1) * (G // 4))
                    nc.sync.dma_start(np_t[:, s, :], npt[:, b0 + k * (G // 4) : b0 + (k + 1) * (G // 4), :])
                return np_t, dp_t, qp_t, s4_t

            def emit_pool(np_t, e_bf, g):
                b0 = g * G
                pl_ps = ppl.tile([D, G], f32, tag="pl")
                se_ps = pse.tile([1, G], f32, tag="se")
                nc.tensor.matmul(
                    se_ps[:], ones_bf[:], e_bf[:], start=True, stop=True,
                    skip_group_check=True,
                )
                for i in range(G):
                    nc.tensor.matmul(
                        pl_ps[:, i : i + 1],
                        np_t[:, i, :],
                        e_bf[:, i : i + 1],
                        start=True,
                        stop=True,
                        skip_group_check=True,
                    )
                se_sb = osb.tile([1, G], f32, tag="sesb")
                nc.vector.tensor_copy(se_sb[:], se_ps[:])
                nc.gpsimd.dma_start(seout[:, b0 : b0 + G], se_sb[:])
                pl_sb = osb.tile([D, G], f32, tag="plsb")
                nc.vector.tensor_copy(pl_sb[:], pl_ps[:])
                nc.gpsimd.dma_start(pout[:, b0 : b0 + G], pl_sb[:])

            loads = emit_loads(0)
            pending = None
            for g in range(NGROUPS):
                np_t, dp_t, qp_t, s4_t = loads
                if g + 1 < NGROUPS:
                    loads = emit_loads(g + 1)

                sc_ps = psc.tile([N, G], f32, tag="sc")
                for i in range(G):
                    nc.tensor.matmul(
                        sc_ps[:, i : i + 1],
                        dp_t[:, i, :],
                        qp_t[:, i : i + 1],
                        start=True,
                        stop=True,
                        skip_group_check=True,
                    )
                # e_pre = scores + sc4 ; e = exp(e_pre) in bf16
                e_pre = epp.tile([N, G], f32, tag="epre")
                nc.vector.scalar_tensor_tensor(
                    out=e_pre[:], in0=sc_ps[:], scalar=1.0, in1=s4_t[:],
                    op0=bypass, op1=add,
                )
                e_bf = ebp.tile([N, G], bf16, tag="ebf")
                nc.scalar.activation(
                    e_bf[:], e_pre[:], func=mybir.ActivationFunctionType.Exp
                )

                if pending is not None:
                    emit_pool(*pending)
                pending = (np_t, e_bf, g)

            emit_pool(*pending)

    nc.compile()
    return nc


def _get_nc():
    global _NC
    if _NC is None:
        _NC = _build()
    return _NC


def prepare_in_maps(near_emb, delta_xy, delta_cs, B_query, W_key):
    """Host-side reformulation: fold W into the query, precompute the
    delta score term, and lay near out in the two PE-friendly layouts."""
    import ml_dtypes

    bf16 = ml_dtypes.bfloat16
    fp8 = ml_dtypes.float8_e3m4

    near_emb = np.asarray(near_emb, dtype=np.float32)
    delta_xy = np.asarray(delta_xy, dtype=np.float32)
    delta_cs = np.asarray(delta_cs, dtype=np.float32)
    B_query = np.asarray(B_query, dtype=np.float32)
    W_key = np.asarray(W_key, dtype=np.float32)

    qp = 0.125 * (B_query @ W_key.T)          # [B, 132]
    sc4 = (
        delta_xy[:, :, 0] * qp[:, 128:129]
        + delta_xy[:, :, 1] * qp[:, 129:130]
        + delta_cs[:, :, 0] * qp[:, 130:131]
        + delta_cs[:, :, 1] * qp[:, 131:132]
    )                                          # [B, N]

    in_maps = []
    for c in range(CORES):
        s = slice(c * PB, (c + 1) * PB)
        nb = near_emb[s]                                   # [PB, N, D]
        nbf = nb.astype(bf16)
        nf8 = nb.astype(fp8)
        in_maps.append(
            {
                "npt": np.ascontiguousarray(nbf.transpose(1, 0, 2)),   # [N, PB, D]
                "dpt": np.ascontiguousarray(nf8.transpose(2, 0, 1)),   # [D, PB, N]
                "qpt": np.ascontiguousarray(qp[s, :128].T).astype(bf16),
                "sc4t": np.ascontiguousarray(sc4[s].T).astype(bf16),
            }
        )
    return in_maps


def finalize(results):
    """Host epilogue: transpose pooled^T back and normalise by sumexp."""
    outs = []
    for c in range(CORES):
        poolT = np.asarray(results[c]["pout"], dtype=np.float32)   # [D, PB]
        se = np.asarray(results[c]["seout"], dtype=np.float32)     # [1, PB]
        outs.append(poolT.T / se.T)
    return np.concatenate(outs, axis=0)


def kernel(near_emb, delta_xy, delta_cs, B_query, W_key, b_key=None, **_ignored):
    from concourse import bass_utils

    in_maps = prepare_in_maps(near_emb, delta_xy, delta_cs, B_query, W_key)
    nc = _get_nc()
    res = bass_utils.run_bass_kernel_spmd(nc, in_maps, core_ids=list(range(CORES)))
    return finalize(res.results)
